# revision 6
# baseline (speedup 1.0000x reference)
"""DTCWT (3-level, concat assembly) Trainium2 Bass kernel.

Strategy (per core = one batch image, 3 channels):
  Each 1-D analysis filter bank (10-tap, stride 2, circular) is a dense matrix
  M [N,N] (rows = lo|hi halves).  2-D separable transform per level:
    stage A (rows):  Y^T = X^T M_r^T  -> PE matmuls, lhsT = X (h on partitions),
                     rhs = M_r^T (dense over h_in k-blocks), out = Y^T (w-major)
    stage B (cols):  Z = Y M_c^T      -> lhsT = Y^T, rhs = M_c^T, out = Z (h-major)
  No transposes anywhere.  fp32r matmuls (full rate at free-dim>=256, ~1e-4 rel).
  x/2 and the dual-tree combine 1/sqrt(2) are folded into the level-0 filter
  matrices (scale 2^-3/4 each); combine is then a plain add/sub pair on DVE
  (one operand PSUM, the other bounced through SBUF -- PSUM has 1 DVE port);
  the final lowpass gets *sqrt(2) at copy-out.
  Channel de-interleave is free: stage-A lhsT reads X with a c-strided AP.
  Outputs are staged in SBUF c-interleaved and DMA'd in large contiguous rows.
"""
import sys
import numpy as np

for _p in ("/opt/trn_rl_repo", "/opt/pypackages"):
    if _p not in sys.path:
        sys.path.append(_p)

import concourse.bass as bass  # noqa: E402,F401
import concourse.mybir as mybir  # noqa: E402
from concourse import bacc  # noqa: E402
from concourse.tile import TileContext  # noqa: E402
from concourse.bass_utils import run_bass_kernel_spmd  # noqa: E402

N_CORES = 8
H = W = 512
C = 3
F32R = mybir.dt.float32r
F32 = mybir.dt.float32
SQRT2 = float(np.sqrt(2.0))

# combine pairs: first member gets (a+b), second gets (a-b)
PAIRS = (((0, 0), (1, 1)), ((0, 1), (1, 0)))


def _build_M(f_lo, f_hi, N, scale=1.0):
    """Analysis matrix, rows 0..N/2-1 lo, N/2.. hi: r[i] = sum_s f[s] x[(2i-s+5) mod N]."""
    M = np.zeros((N, N), dtype=np.float64)
    for half, f in enumerate((f_lo, f_hi)):
        for i in range(N // 2):
            for s in range(10):
                j = (2 * i - s + 5) % N
                M[half * (N // 2) + i, j] += float(f[s]) * scale
    return M


def _wc(ap, c):
    """Channel-c plane view of a [P, W*C] staging tile -> [P, W] stride-C AP."""
    return ap.rearrange("p (w c) -> p w c", c=C)[:, :, c]


def _build_module():
    nc = bacc.Bacc("TRN2", target_bir_lowering=False, debug=False)
    x = nc.declare_dram_parameter("x", [H, W, C], F32R, isOutput=False)
    mt0p = nc.declare_dram_parameter("mt0", [2, 512, 512], F32R, isOutput=False)
    mt1p = nc.declare_dram_parameter("mt1", [2, 256, 256], F32R, isOutput=False)
    mt2p = nc.declare_dram_parameter("mt2", [128, 256], F32R, isOutput=False)
    outp = nc.declare_dram_parameter("out", [2 * H, 2 * W, C], F32, isOutput=True)

    def out_slice(r0, r1, c0, c1):
        return outp[r0:r1, c0:c1, :].rearrange("p w c -> p (w c)")

    with TileContext(nc) as tc:
        with (
            tc.tile_pool(name="const", bufs=1) as cp,
            tc.tile_pool(name="stg", bufs=2) as sp,
            tc.tile_pool(name="psA", bufs=2, space="PSUM") as psA,
            tc.tile_pool(name="psB", bufs=4, space="PSUM") as psB,
        ):
            lvl0 = tc.tile_pool(name="lvl0", bufs=1)
            lvl0p = lvl0.__enter__()

            # ---- load inputs / constants -------------------------------------
            xt = []
            for hb in range(4):
                t = lvl0p.tile([128, W * C], F32R, tag=f"x{hb}", name=f"x{hb}")
                nc.sync.dma_start(out=t[:], in_=x[128 * hb:128 * (hb + 1)].rearrange("p w c -> p (w c)"))
                xt.append(t)
            mt0 = []
            for t_ in range(2):
                m = cp.tile([128, 4 * 512], F32R, tag=f"mt0_{t_}", name=f"mt0_{t_}")
                for kb in range(4):
                    nc.sync.dma_start(out=m[:, 512 * kb:512 * (kb + 1)],
                                      in_=mt0p[t_, 128 * kb:128 * (kb + 1), :])
                mt0.append(m)
            mt1 = []
            for t_ in range(2):
                m = cp.tile([128, 2 * 256], F32R, tag=f"mt1_{t_}", name=f"mt1_{t_}")
                for kb in range(2):
                    nc.sync.dma_start(out=m[:, 256 * kb:256 * (kb + 1)],
                                      in_=mt1p[t_, 128 * kb:128 * (kb + 1), :])
                mt1.append(m)
            mt2 = cp.tile([128, 256], F32R, tag="mt2", name="mt2")
            nc.sync.dma_start(out=mt2[:], in_=mt2p[:, :])

            # ---- LEVEL 0 ------------------------------------------------------
            # stage A: Y^T[m][c]  [w 4x128, h_out 512]  (cols 512*wb + h_out)
            yt = {}
            for m in range(2):
                for c in range(C):
                    dst = lvl0p.tile([128, 2048], F32R, tag=f"yt{m}{c}", name=f"yt{m}{c}")
                    yt[(m, c)] = dst
                    for half in range(2):
                        pa = psA.tile([128, 1024], F32, tag="pa", name="pa")
                        for wb2 in range(2):
                            wb = 2 * half + wb2
                            lhs_w = slice(128 * wb, 128 * (wb + 1))
                            for kb in range(4):
                                nc.tensor.matmul(
                                    pa[:, 512 * wb2:512 * (wb2 + 1)],
                                    _wc(xt[kb], c)[:, lhs_w],
                                    mt0[m][:, 512 * kb:512 * (kb + 1)],
                                    start=(kb == 0), stop=(kb == 3))
                        nc.scalar.copy(dst[:, 1024 * half:1024 * (half + 1)], pa[:])

            # stage B + combine.
            # llv[(mn,c)][hb] : [128,256] (or wider) f32r AP feeding level-1 stage A.
            llv = {}
            for (p, q) in PAIRS:
                for c in range(C):
                    llv[(p, c)] = [cp.tile([128, 256], F32R, tag=f"llp{p[0]}{p[1]}{c}h{hb}",
                                           name=f"llp{p[0]}{p[1]}{c}h{hb}")[:]
                                   for hb in range(2)]
                    # q member: full Z_q rows land in sbuf anyway (combine needs an
                    # SBUF operand); the lo-cols double as the level-1 input.
                    llv[(q, c)] = [cp.tile([128, 512], F32R, tag=f"zqf{q[0]}{q[1]}{c}h{hb}",
                                           name=f"zqf{q[0]}{q[1]}{c}h{hb}")[:]
                                   for hb in range(2)]

            def stageB_mm(z, t, hb, c):
                """accumulate z[128,512] over the 4 w-blocks for tree t=(m,n)."""
                for wb in range(4):
                    nc.tensor.matmul(
                        z[:],
                        yt[(t[0], c)][:, 512 * wb + 128 * hb: 512 * wb + 128 * (hb + 1)],
                        mt0[t[1]][:, 512 * wb:512 * (wb + 1)],
                        start=(wb == 0), stop=(wb == 3))

            for (p, q) in PAIRS:
                for hb in range(4):
                    if hb < 2:
                        stw0 = {t: sp.tile([128, 256 * C], F32, tag="stgw0", name="stgw0")
                                for t in (p, q)}
                    else:
                        stw12 = {t: sp.tile([128, 512 * C], F32, tag="stgw12", name="stgw12")
                                 for t in (p, q)}
                    for c in range(C):
                        zp = psB.tile([128, 512], F32, tag="zb", name="zb")
                        zq = psB.tile([128, 512], F32, tag="zb", name="zb")
                        stageB_mm(zp, p, hb, c)
                        stageB_mm(zq, q, hb, c)
                        if hb < 2:
                            zqf = llv[(q, c)][hb]          # persistent [128,512]
                        else:
                            zqf = sp.tile([128, 512], F32R, tag="zqf_t", name="zqf_t")[:]
                        nc.scalar.copy(zqf, zq[:])
                        zqs = zqf.bitcast(F32)
                        if hb < 2:
                            nc.scalar.copy(llv[(p, c)][hb], zp[:, 0:256])
                            nc.vector.tensor_add(_wc(stw0[p], c), zp[:, 256:512], zqs[:, 256:512])
                            nc.vector.tensor_sub(_wc(stw0[q], c), zp[:, 256:512], zqs[:, 256:512])
                        else:
                            nc.vector.tensor_add(_wc(stw12[p], c), zp[:], zqs)
                            nc.vector.tensor_sub(_wc(stw12[q], c), zp[:], zqs)
                    for t in (p, q):
                        r0, c0 = 512 * t[0], 512 * t[1]
                        if hb < 2:
                            nc.sync.dma_start(
                                out=out_slice(r0 + 128 * hb, r0 + 128 * (hb + 1), c0 + 256, c0 + 512),
                                in_=stw0[t][:])
                        else:
                            nc.sync.dma_start(
                                out=out_slice(r0 + 128 * hb, r0 + 128 * (hb + 1), c0, c0 + 512),
                                in_=stw12[t][:])

            lvl0.__exit__(None, None, None)

            # ---- LEVEL 1 ------------------------------------------------------
            lvl1 = tc.tile_pool(name="lvl1", bufs=1)
            lvl1p = lvl1.__enter__()
            trees = [t for pr in PAIRS for t in pr]
            yt1 = {}
            for mn in trees:
                for c in range(C):
                    dst = lvl1p.tile([128, 512], F32R, tag=f"yt1_{mn[0]}{mn[1]}{c}",
                                     name=f"yt1_{mn[0]}{mn[1]}{c}")
                    yt1[(mn, c)] = dst
                    pa = psA.tile([128, 1024], F32, tag="pa", name="pa")
                    for wb in range(2):
                        for kb in range(2):
                            nc.tensor.matmul(
                                pa[:, 256 * wb:256 * (wb + 1)],
                                llv[(mn, c)][kb][:, 128 * wb:128 * (wb + 1)],
                                mt1[mn[0]][:, 256 * kb:256 * (kb + 1)],
                                start=(kb == 0), stop=(kb == 1))
                    nc.scalar.copy(dst[:], pa[:, 0:512])

            # ll1v[(mn,c)] : [128,128] (or wider) f32r AP feeding level-2 stage A.
            ll1v = {}
            for (p, q) in PAIRS:
                for c in range(C):
                    ll1v[(p, c)] = cp.tile([128, 128], F32R, tag=f"ll1p{p[0]}{p[1]}{c}",
                                           name=f"ll1p{p[0]}{p[1]}{c}")[:]
                    ll1v[(q, c)] = cp.tile([128, 256], F32R, tag=f"zq1f{q[0]}{q[1]}{c}",
                                           name=f"zq1f{q[0]}{q[1]}{c}")[:]

            for (p, q) in PAIRS:
                for hb in range(2):
                    if hb == 0:
                        st0 = {t: sp.tile([128, 128 * C], F32, tag="stg1w0", name="stg1w0") for t in (p, q)}
                    else:
                        st12 = {t: sp.tile([128, 256 * C], F32, tag="stg1w12", name="stg1w12") for t in (p, q)}
                    for c in range(C):
                        zp = psB.tile([128, 512], F32, tag="zb", name="zb")
                        zq = psB.tile([128, 512], F32, tag="zb", name="zb")
                        for (z, t) in ((zp, p), (zq, q)):
                            for wb in range(2):
                                nc.tensor.matmul(
                                    z[:, 0:256],
                                    yt1[(t, c)][:, 256 * wb + 128 * hb: 256 * wb + 128 * (hb + 1)],
                                    mt1[t[1]][:, 256 * wb:256 * (wb + 1)],
                                    start=(wb == 0), stop=(wb == 1))
                        if hb == 0:
                            zqf = ll1v[(q, c)]             # persistent [128,256]
                        else:
                            zqf = sp.tile([128, 256], F32R, tag="zq1f_t", name="zq1f_t")[:]
                        nc.scalar.copy(zqf, zq[:, 0:256])
                        zqs = zqf.bitcast(F32)
                        if hb == 0:
                            nc.scalar.copy(ll1v[(p, c)], zp[:, 0:128])
                            nc.vector.tensor_add(_wc(st0[p], c), zp[:, 128:256], zqs[:, 128:256])
                            nc.vector.tensor_sub(_wc(st0[q], c), zp[:, 128:256], zqs[:, 128:256])
                        else:
                            nc.vector.tensor_add(_wc(st12[p], c), zp[:, 0:256], zqs)
                            nc.vector.tensor_sub(_wc(st12[q], c), zp[:, 0:256], zqs)
                    for t in (p, q):
                        r0, c0 = 512 * t[0], 512 * t[1]
                        if hb == 0:
                            nc.sync.dma_start(out=out_slice(r0, r0 + 128, c0 + 128, c0 + 256),
                                              in_=st0[t][:])
                        else:
                            nc.sync.dma_start(out=out_slice(r0 + 128, r0 + 256, c0, c0 + 256),
                                              in_=st12[t][:])

            lvl1.__exit__(None, None, None)

            # ---- LEVEL 2 ------------------------------------------------------
            lvl2 = tc.tile_pool(name="lvl2", bufs=1)
            lvl2p = lvl2.__enter__()
            y2 = {}
            for mn in trees:
                for c in range(C):
                    dst = lvl2p.tile([128, 128], F32R, tag=f"y2_{mn[0]}{mn[1]}{c}",
                                     name=f"y2_{mn[0]}{mn[1]}{c}")
                    y2[(mn, c)] = dst
                    pa = psA.tile([128, 1024], F32, tag="pa", name="pa")
                    nc.tensor.matmul(pa[:, 0:256], ll1v[(mn, c)][:, 0:128], mt2[:],
                                     start=True, stop=True)
                    nc.scalar.copy(dst[:], pa[:, 128 * mn[0]:128 * (mn[0] + 1)])

            for (p, q) in PAIRS:
                st2 = {t: sp.tile([128, 128 * C], F32, tag="stg2", name="stg2") for t in (p, q)}
                for c in range(C):
                    zp = psB.tile([128, 512], F32, tag="zb", name="zb")
                    zq = psB.tile([128, 512], F32, tag="zb", name="zb")
                    nc.tensor.matmul(zp[:, 0:256], y2[(p, c)][:], mt2[:], start=True, stop=True)
                    nc.tensor.matmul(zq[:, 0:256], y2[(q, c)][:], mt2[:], start=True, stop=True)
                    op_, oq_ = 128 * p[1], 128 * q[1]
                    zqf = sp.tile([128, 128], F32, tag="zq2f_t", name="zq2f_t")
                    nc.scalar.copy(zqf[:], zq[:, oq_:oq_ + 128])

                    def wv(ap, c=c):
                        return ap.rearrange("p (w c) -> p w c", c=C)[:, :, c]
                    # final lowpass (*sqrt2), not combined
                    nc.scalar.mul(wv(st2[p][0:64, :])[:, 0:64], zp[0:64, op_:op_ + 64], SQRT2)
                    nc.scalar.mul(wv(st2[q][0:64, :])[:, 0:64], zq[0:64, oq_:oq_ + 64], SQRT2)
                    # w0 (rows 0:64, quadrant cols 64:128)
                    nc.vector.tensor_add(wv(st2[p][0:64, :])[:, 64:128],
                                         zp[0:64, op_ + 64:op_ + 128], zqf[0:64, 64:128])
                    nc.vector.tensor_sub(wv(st2[q][0:64, :])[:, 64:128],
                                         zp[0:64, op_ + 64:op_ + 128], zqf[0:64, 64:128])
                    # w1|w2 (rows 64:128, quadrant cols 0:128)
                    nc.vector.tensor_add(wv(st2[p][64:128, :]),
                                         zp[64:128, op_:op_ + 128], zqf[64:128, :])
                    nc.vector.tensor_sub(wv(st2[q][64:128, :]),
                                         zp[64:128, op_:op_ + 128], zqf[64:128, :])
                for t in (p, q):
                    r0, c0 = 512 * t[0], 512 * t[1]
                    nc.sync.dma_start(out=out_slice(r0, r0 + 128, c0, c0 + 128), in_=st2[t][:])
            lvl2.__exit__(None, None, None)

    nc.compile()
    return nc


_NC_CACHE = None


def _get_nc():
    global _NC_CACHE
    if _NC_CACHE is None:
        _NC_CACHE = _build_module()
    return _NC_CACHE


def _filter_mats(Faf, af):
    s0 = 2.0 ** (-0.75)
    mt0 = np.stack([_build_M(Faf[t, 0], Faf[t, 1], 512, s0).T for t in range(2)])
    mt1 = np.stack([_build_M(af[t, 0], af[t, 1], 256).T for t in range(2)])
    mt2 = np.concatenate([_build_M(af[t, 0], af[t, 1], 128).T for t in range(2)], axis=1)
    return (np.ascontiguousarray(mt0, dtype=np.float32),
            np.ascontiguousarray(mt1, dtype=np.float32),
            np.ascontiguousarray(mt2, dtype=np.float32))


def _run(x, Faf, af, trace=False):
    nc = _get_nc()
    mt0, mt1, mt2 = _filter_mats(np.asarray(Faf), np.asarray(af))
    xs = np.ascontiguousarray(np.asarray(x, dtype=np.float32))
    in_maps = [{"x": xs[b], "mt0": mt0, "mt1": mt1, "mt2": mt2} for b in range(N_CORES)]
    br = run_bass_kernel_spmd(nc, in_maps, core_ids=list(range(N_CORES)), trace=trace)
    out = np.stack([br.results[b]["out"] for b in range(N_CORES)])
    return out, br


def kernel(x, Faf, af, level):
    assert int(level) == 3, f"only level=3 supported, got {level}"
    out, _ = _run(x, Faf, af, trace=False)
    return out
